# revision 1
# baseline (speedup 1.0000x reference)
"""Trainium2 Bass kernel for nn_KernelAttnCoef (linear attention variant).

Math (per batch b, head h):
    A_h = q_h @ k_h^T                      # [n, n]   (n=256, d=16)
    qk_sum_h[k] = sum_l A_h[k, l]          # normalizer (== q_h . sum_l k_h)
    att_h = (A_h / qk_sum_h[:, None]) @ v_h    # v_h: [n(l), n(t), 8]
    out[b, k, t, 8h+j] = att_h[k, t, j]

Sharding: batch-parallel, core b <- batch b. The tiny normalized
A^T (2MB/core) is computed on the host; the device streams the 16MB v
through the tensor engine against per-head A^T weights and writes the
16MB output, both as large contiguous DMAs (memory-bound regime).
"""

import numpy as np

B = 8
N = 256
H = 8
DQK = 16
DVH = 8
DV = 64
EPS = 1e-05

MODE = "f32"  # "f32" | "f32r" | "bf16x3"
TC = 64       # t-tile size (TC*DVH = 512 = one PSUM bank of fp32)

_cache = {}


def _build(mode):
    from contextlib import ExitStack

    import concourse.tile as tile
    from concourse import bacc, mybir

    nc = bacc.Bacc("TRN2", target_bir_lowering=False, debug=False, num_devices=8)
    if mode == "bf16x3":
        dt_in, n_planes = mybir.dt.bfloat16, 2
        terms = [(0, 0), (0, 1), (1, 0)]  # (at_plane, v_plane): hh + hl + lh
    elif mode == "f32r":
        dt_in, n_planes = mybir.dt.float32r, 1
        terms = [(0, 0)]
    else:
        dt_in, n_planes = mybir.dt.float32, 1
        terms = [(0, 0)]

    at_d = nc.dram_tensor(
        "at", [n_planes, 2, 128, H * N], dt_in, kind="ExternalInput"
    ).ap()
    v_d = nc.dram_tensor(
        "v", [n_planes, N, N * DV], dt_in, kind="ExternalInput"
    ).ap()
    out_d = nc.dram_tensor(
        "out", [N, N * DV], mybir.dt.float32, kind="ExternalOutput"
    ).ap()

    n_tc = N // TC
    FW = TC * DV  # free width of one (lc) v tile / out tile

    with tile.TileContext(nc) as tc:
        with ExitStack() as ctx:
            at_pool = ctx.enter_context(tc.tile_pool(name="at", bufs=1))
            v_pool = ctx.enter_context(tc.tile_pool(name="v", bufs=2))
            o_pool = ctx.enter_context(tc.tile_pool(name="o", bufs=2))
            ps_pool = ctx.enter_context(
                tc.tile_pool(name="ps", bufs=8, space="PSUM")
            )

            at_sb = {}
            for p in range(n_planes):
                for lc in range(2):
                    t = at_pool.tile([128, H * N], dt_in, tag=f"at{p}{lc}")
                    nc.sync.dma_start(out=t[:], in_=at_d[p, lc])
                    at_sb[p, lc] = t

            nmm = 2 * len(terms)
            for tci in range(n_tc):
                vt = {}
                for p in range(n_planes):
                    for lc in range(2):
                        t = v_pool.tile([128, FW], dt_in, tag=f"v{p}{lc}")
                        nc.sync.dma_start(
                            out=t[:],
                            in_=v_d[
                                p,
                                lc * 128 : (lc + 1) * 128,
                                tci * FW : (tci + 1) * FW,
                            ],
                        )
                        vt[p, lc] = t
                for kc in range(2):
                    ot = o_pool.tile([128, FW], mybir.dt.float32, tag=f"o{kc}")
                    ov = ot[:].rearrange("p (t c) -> p t c", c=DV)
                    for h in range(H):
                        ps = ps_pool.tile([128, TC * DVH], mybir.dt.float32, tag="ps")
                        i = 0
                        for lc in range(2):
                            for (ap_, vp) in terms:
                                nc.tensor.matmul(
                                    ps[:],
                                    lhsT=at_sb[ap_, lc][
                                        :, h * N + kc * 128 : h * N + kc * 128 + 128
                                    ],
                                    rhs=vt[vp, lc][:]
                                    .rearrange("p (t c) -> p t c", c=DV)[
                                        :, :, h * DVH : (h + 1) * DVH
                                    ],
                                    start=(i == 0),
                                    stop=(i == nmm - 1),
                                )
                                i += 1
                        nc.vector.tensor_copy(
                            out=ov[:, :, h * DVH : (h + 1) * DVH],
                            in_=ps[:].rearrange("p (t j) -> p t j", j=DVH),
                        )
                    nc.sync.dma_start(
                        out=out_d[
                            kc * 128 : (kc + 1) * 128, tci * FW : (tci + 1) * FW
                        ],
                        in_=ot[:],
                    )
    nc.compile()
    return nc


def _get_nc(mode=None):
    mode = mode or MODE
    if mode not in _cache:
        _cache[mode] = _build(mode)
    return _cache[mode]


def _prep_inputs(query, key, value, mode=None):
    """Host prep: per-core (per-batch) input maps."""
    mode = mode or MODE
    in_maps = []
    for b in range(B):
        qb = np.asarray(query[b], np.float64)
        kb = np.asarray(key[b], np.float64)
        at = np.empty((2, 128, H, N), np.float64)  # [lc, l, h, k]
        for h in range(H):
            qh = qb[:, h * DQK : (h + 1) * DQK]
            kh = kb[:, h * DQK : (h + 1) * DQK]
            A = qh @ kh.T  # [k, l]
            qk = A.sum(axis=1).astype(np.float32)  # matches reference fp32 zero-guard
            qk = np.where(qk == 0, np.float32(EPS), qk).astype(np.float64)
            atp = (A / qk[:, None]).T  # [l, k]
            at[0, :, h, :] = atp[:128]
            at[1, :, h, :] = atp[128:]
        at = at.reshape(2, 128, H * N)
        vb = np.asarray(value[b], np.float32).reshape(N, N * DV)
        if mode == "bf16x3":
            import ml_dtypes

            bf16 = ml_dtypes.bfloat16
            a32 = at.astype(np.float32)
            ah = a32.astype(bf16)
            al = (a32 - ah.astype(np.float32)).astype(bf16)
            vh = vb.astype(bf16)
            vl = (vb - vh.astype(np.float32)).astype(bf16)
            in_maps.append(
                {"at": np.stack([ah, al]), "v": np.stack([vh, vl])}
            )
        else:
            in_maps.append(
                {"at": at.astype(np.float32)[None], "v": vb[None]}
            )
    return in_maps


def kernel(query, key, value):
    from concourse.bass_utils import run_bass_kernel_spmd

    nc = _get_nc()
    in_maps = _prep_inputs(query, key, value)
    res = run_bass_kernel_spmd(nc, in_maps, list(range(B)))
    return np.stack(
        [res.results[b]["out"].reshape(N, N, DV) for b in range(B)]
    )
